# revision 8
# baseline (speedup 1.0000x reference)
"""Trainium2 Bass kernel for BConvAttention2d.

v5: row-parity final conv with half-split bsa tiles.

Host prep: input is sign()ed, patch-padded and fp8-cast on the host;
output is written f16 (exact: conv outputs are integers <= 576) and
upcast on the host.

Device, per core: 4 images as 2 pairs (2 images x 64ch -> 128 parts),
pairs alternate group-by-group so cross-engine results age a full
iteration before dependents are issued.
  1. DMA binp fp8 [128, 8p, 18, 20] (zero patch borders baked by host).
  2. Depthwise 3x3 per (channel, patch): diag-matrix weights on PE,
     DoubleRow fp8 pair-matmuls, 4 pairs + 1 single per patch, N=256.
  3. DVE clip(psum,-1,1) == sign (psum is integer) -> per-image bsa
     tiles holding TWO row-shifted copies (partition half s reads image
     row y+s); SBUF->SBUF DMAs fill the shifted halves.  Each image's
     bsa is SPLIT into top/bottom half-tiles so the final conv's
     whole-tile dependencies are always >= 2 iterations old.
  4. Final conv as 3 accumulating DoubleRow matmuls per 8-row tile:
     K = (row-shift s x 64ic), DR pair = row-delta-2, M = (row-parity r
     x 64oc), one pass per horizontal tap v: all 9 taps in 3 passes
     with only 6 global weight matrices.
  5. ACT evicts PSUM->SBUF f16 (evictions and signs on separate
     engines so neither queues behind the other); DMA out.
"""

import numpy as np
import ml_dtypes

import concourse.bass as bass
import concourse.mybir as mybir
from concourse.tile import TileContext
from concourse.ap import AP
from concourse.bass_utils import run_bass_kernel_spmd

# ---- problem constants (hardcoded per contract) ----
B, C, H, W = 32, 64, 128, 128
N_CORES = 8
B_CORE = B // N_CORES          # 4 images per core
N_PAIRS = B_CORE // 2
PATCH = 16
NP_SIDE = H // PATCH           # 8x8 patch grid
TAPS = 9
FP8 = mybir.dt.float8e4
F16 = mybir.dt.float16
F32 = mybir.dt.float32
DR = mybir.MatmulPerfMode.DoubleRow

# depthwise tap pairing: 4 DoubleRow pairs + 1 single (taps as (u, v))
DW_PAIRS = [((0, 0), (0, 2)), ((1, 0), (1, 2)), ((2, 0), (2, 2)),
            ((0, 1), (2, 1))]
DW_SINGLE = (1, 1)

_CACHED_NC = None


def _pair_ap(a, delta):
    """Insert a [delta, 2] DoubleRow pair dim after the partition dim."""
    dims = [list(p) for p in a.ap]
    return AP(tensor=a.tensor, offset=a.offset,
              ap=[dims[0], [delta, 2]] + dims[1:])


def _fc_rhs(bt, row, v):
    """rhs AP for a final-conv tile on a bsa half-tile starting at stored
    row `row`, tap column v: [part, pair j (row +2), y0 (row +2), x]."""
    a = bt[:, row, v:v + W]
    dims = [list(p) for p in a.ap]
    return AP(tensor=a.tensor, offset=a.offset,
              ap=[dims[0], [260, 2], [260, 4], dims[1]])


def _split_multiwaits(nc):
    """walrus codegen in this toolchain accepts only ONE embedded sync wait
    per instruction; hoist extras onto preceding NOPs on the same engine."""
    for f in nc.m.functions:
        for blk in f.blocks:
            new_insts = []
            for inst in blk.instructions:
                si = inst.sync_info
                if si is not None and len(si.on_wait) > 1:
                    waits = list(si.on_wait)
                    for w in waits[:-1]:
                        nop = mybir.InstNoOp(
                            name=nc.get_next_instruction_name(), ins=[], outs=[]
                        )
                        nop.engine = inst.engine
                        nop.sync_info = mybir.SyncInfo(on_wait=[w], on_update=[])
                        new_insts.append(nop)
                    inst.sync_info = mybir.SyncInfo(
                        on_wait=[waits[-1]], on_update=list(si.on_update)
                    )
                new_insts.append(inst)
            blk.instructions[:] = new_insts


def _tset(q):
    """final-conv tiles whose bsa rows are complete after group q's copies."""
    if q == 0:
        return [0]
    if q < 7:
        return [2 * q - 1, 2 * q]
    return [13, 14, 15]


def _build_nc():
    nc = bass.Bass()
    # host-signed, patch-padded fp8 input: [pair, 128, g, p, 18, 20]
    xs = nc.declare_dram_parameter(
        "xs", [N_PAIRS, 128, NP_SIDE, NP_SIDE, 18, 20], FP8, isOutput=False
    )
    # diag depthwise weights: [k, group, patch, slot, m]; slots 0..7 are the
    # 4 DoubleRow pairs, slot 8 the single tap
    wd = nc.declare_dram_parameter(
        "wd", [128, NP_SIDE, NP_SIDE, TAPS, 128], FP8, isOutput=False
    )
    # row-parity final conv weights: [k, img parity, v, j, m]
    wfcp = nc.declare_dram_parameter("wfcp", [128, 2, 3, 2, 128], FP8,
                                     isOutput=False)
    y = nc.declare_dram_parameter("y", [B_CORE, C, H, W], F16, isOutput=True)

    with TileContext(nc) as tc:
        with (
            tc.tile_pool(name="persist", bufs=1) as persist,
            tc.tile_pool(name="inp", bufs=4) as inpool,
            tc.tile_pool(name="outp", bufs=6) as outpool,
            tc.tile_pool(name="dwps", bufs=2, space="PSUM") as dwpsum,
            tc.tile_pool(name="cvps", bufs=3, space="PSUM") as cvpsum,
            tc.tile_pool(name="wmps", bufs=1, space="PSUM") as wmpsum,
        ):
            wfcp_sb = persist.tile([128, 2, 3, 2, 128], FP8)
            wd_sb = persist.tile([128, NP_SIDE, NP_SIDE, TAPS, 128], FP8)
            # per-image bsa in two half-tiles: TOP = stored rows 0..64,
            # BOT = stored rows 64..129 (stored row 64 duplicated).
            # Even image: shift-0 copy in partitions 0-63; odd image:
            # shift-0 in partitions 64-127 (matches DW psum lanes).
            top = [persist.tile([128, 65, 130], FP8, name=f"top{i}")
                   for i in range(B_CORE)]
            bot = [persist.tile([128, 66, 130], FP8, name=f"bot{i}")
                   for i in range(B_CORE)]

            nc.sync.dma_start(out=wfcp_sb, in_=wfcp[:])

            for b in range(B_CORE):
                nc.gpsimd.memset(top[b][:, 0, :], 0.0)       # image pad row
                nc.gpsimd.memset(top[b][:, :, 0], 0.0)
                nc.gpsimd.memset(top[b][:, :, 129], 0.0)
                nc.gpsimd.memset(bot[b][:, 64:66, :], 0.0)   # pad rows
                nc.gpsimd.memset(bot[b][:, :, 0], 0.0)
                nc.gpsimd.memset(bot[b][:, :, 129], 0.0)

            def fc_tile(b, t):
                # output rows 8t..8t+7 of image b via 3 DR passes (v taps)
                par = b % 2
                bt, row = (top[b], 8 * t) if t <= 7 else (bot[b], 8 * t - 64)
                pt = cvpsum.tile([128, 4, W], F32, name="cvp")
                for v in range(3):
                    nc.tensor.matmul(
                        pt, lhsT=wfcp_sb[:, par, v], rhs=_fc_rhs(bt, row, v),
                        start=(v == 0), stop=(v == 2), perf_mode=DR,
                    )
                ot = outpool.tile([128, 4, W], F16, name="ot")
                nc.scalar.copy(out=ot, in_=pt)
                # partition r*64+oc -> y[b, oc, 8t + 2*y0 + r, x]
                a = y[b, :, 8 * t:8 * t + 8, :]
                c_str, c_n = a.ap[0]
                h_str, _ = a.ap[1]
                w_str, w_n = a.ap[2]
                for r in range(2):
                    nc.sync.dma_start(
                        out=AP(tensor=a.tensor, offset=a.offset + r * h_str,
                               ap=[[c_str, c_n], [2 * h_str, 4],
                                   [w_str, w_n]]),
                        in_=ot[64 * r:64 * r + 64],
                    )

            def load_binp(pi, g):
                tl = inpool.tile([128, NP_SIDE, 18, 20], FP8, name="binp")
                nc.sync.dma_start(out=tl, in_=xs[pi, :, g])
                return tl

            def dw_halfgroup(binp, g, sb):
                pg = dwpsum.tile([128, 4, PATCH, PATCH], F32, name="pg")
                for pj in range(4):
                    p = 4 * sb + pj
                    for kk, (ta, tb) in enumerate(DW_PAIRS):
                        ua, va = ta
                        dlt = (tb[0] - ua) * 20 + (tb[1] - va)
                        rhs = _pair_ap(
                            binp[:, p, ua:ua + 16, 1 + va:17 + va], dlt
                        )
                        nc.tensor.matmul(
                            pg[:, pj],
                            lhsT=wd_sb[:, g, p, 2 * kk:2 * kk + 2, :],
                            rhs=rhs,
                            start=(kk == 0), stop=False, perf_mode=DR,
                        )
                    u, v = DW_SINGLE
                    nc.tensor.matmul(
                        pg[:, pj], lhsT=wd_sb[:, g, p, 8, :],
                        rhs=binp[:, p, u:u + 16, 1 + v:17 + v],
                        start=False, stop=True,
                    )
                return pg

            def signs(pg, be, bo, g, sb):
                # clip(psum,-1,1) == sign for integer psum.  Both images on
                # DVE so ACT (evictions) never queues behind these.
                if g <= 3:
                    dst_e, dst_o = top[be], top[bo]
                    r0 = 1 + PATCH * g
                else:
                    dst_e, dst_o = bot[be], bot[bo]
                    r0 = 1 + PATCH * (g - 4)
                rows = slice(r0, r0 + PATCH)
                cols = slice(1 + 64 * sb, 65 + 64 * sb)
                for dst, half, img in ((dst_e, slice(0, 64), 0),
                                       (dst_o, slice(64, 128), 1)):
                    nc.vector.tensor_scalar(
                        out=dst[half, rows, cols].rearrange(
                            "n i (pc j) -> n pc i j", pc=4),
                        in0=pg[half],
                        scalar1=-1.0, scalar2=1.0,
                        op0=mybir.AluOpType.max, op1=mybir.AluOpType.min,
                    )

            def copies(be, bo, g):
                # fill shift-1 halves (stored row ry = image-padded row ry+1)
                # and the duplicated boundary rows between TOP and BOT
                for b in (be, bo):
                    s0 = slice(0, 64) if b % 2 == 0 else slice(64, 128)
                    s1 = slice(64, 128) if b % 2 == 0 else slice(0, 64)
                    if g <= 3:
                        nc.sync.dma_start(
                            out=top[b][s1, PATCH * g:PATCH * (g + 1), :],
                            in_=top[b][s0, 1 + PATCH * g:1 + PATCH * (g + 1), :],
                        )
                        if g == 3:   # shift-0 stored row 64 -> BOT row 0
                            nc.sync.dma_start(out=bot[b][s0, 0, :],
                                              in_=top[b][s0, 64, :])
                    else:
                        gb = g - 4
                        nc.sync.dma_start(
                            out=bot[b][s1, PATCH * gb:PATCH * (gb + 1), :],
                            in_=bot[b][s0, 1 + PATCH * gb:1 + PATCH * (gb + 1), :],
                        )
                        if g == 4:   # shift-1 stored row 64 -> TOP row 64
                            nc.sync.dma_start(out=top[b][s1, 64, :],
                                              in_=bot[b][s0, 1, :])

            # PE warmup: keep the tensor engine busy through the input-DMA
            # wait so the HAM clock gate is released (2.4GHz) when the real
            # matmul stream starts
            warm_ps = wmpsum.tile([128, 4, W], F32, name="warm")
            for _ in range(96):
                nc.tensor.matmul(warm_ps[:, 0, 0:64],
                                 lhsT=wfcp_sb[:, 0, 0, 0, :],
                                 rhs=wfcp_sb[:, 0, 0, 0, 0:64],
                                 start=True, stop=True)

            # alternate pairs group-by-group: each pair's sign/copy results
            # age a full iteration (~5us of the other pair's work) before
            # anything depending on them is issued
            seq = [(pi, g) for g in range(NP_SIDE) for pi in range(N_PAIRS)]
            binp_tiles = {0: load_binp(0, 0)}
            # first weight group split so patch 0's weights land early
            nc.sync.dma_start(out=wd_sb[:, 0, 0:2], in_=wd[:, 0, 0:2])
            nc.sync.dma_start(out=wd_sb[:, 0, 2:8], in_=wd[:, 0, 2:8])
            binp_tiles[1] = load_binp(1, 0)
            for si, (pi, g) in enumerate(seq):
                be, bo = 2 * pi, 2 * pi + 1     # even/odd image of this pair
                binp = binp_tiles.pop(si)
                if si + 2 < len(seq):
                    npi, ng = seq[si + 2]
                    binp_tiles[si + 2] = load_binp(npi, ng)
                if pi == 0 and g < NP_SIDE - 1:
                    nc.sync.dma_start(out=wd_sb[:, g + 1], in_=wd[:, g + 1])

                pg0 = dw_halfgroup(binp, g, 0)
                if g >= 2:
                    for im in (be, bo):
                        for tt in _tset(g - 2):
                            fc_tile(im, tt)
                if si > 0:
                    ppi, pgr = seq[si - 1]
                    copies(2 * ppi, 2 * ppi + 1, pgr)
                signs(pg0, be, bo, g, 0)
                pg1 = dw_halfgroup(binp, g, 1)
                signs(pg1, be, bo, g, 1)

            # tail: last copies, then both pairs' remaining tiles (pair 0's
            # fill the copy latency of pair 1's)
            copies(2, 3, NP_SIDE - 1)
            for pi in range(N_PAIRS):
                for im in (2 * pi, 2 * pi + 1):
                    for tt in _tset(6) + _tset(7):
                        fc_tile(im, tt)

    _split_multiwaits(nc)
    return nc


def _host_weights(patch_filters, output_filters):
    to8 = lambda a: np.ascontiguousarray(a).astype(ml_dtypes.float8_e4m3fn)
    ar = np.arange(128)
    # depthwise diag tiles: wd[c+64s, g, p, slot, m] = w[c, 8g+p, tap] iff m==c+64s
    pfs = np.sign(np.asarray(patch_filters, np.float32))[:, :, 0]  # [c, P, 3, 3]
    pf2 = np.concatenate([pfs, pfs], axis=0)                       # [128, P, 3, 3]
    taps = [t for pr in DW_PAIRS for t in pr] + [DW_SINGLE]        # 9 slot taps
    wd = np.zeros((128, NP_SIDE, NP_SIDE, TAPS, 128), np.float32)
    for s, (u, v) in enumerate(taps):
        wd[ar, :, :, s, ar] = pf2[:, :, u, v].reshape(128, NP_SIDE, NP_SIDE)
    # row-parity final conv: wfcp[64*kblk+ic, par, v, j, 64*r+oc]
    #   = sign(of)[oc, ic, u=2j+s-r, v], kblk = s for even images (par 0),
    #     1-s for odd images (shift-0 half lives in partitions 64-127)
    ofs = np.sign(np.asarray(output_filters, np.float32))          # [o, i, 3, 3]
    wfcp = np.zeros((128, 2, 3, 2, 128), np.float32)
    for par in range(2):
        for v in range(3):
            for s in range(2):
                kblk = s if par == 0 else 1 - s
                for j in range(2):
                    for r in range(2):
                        u = 2 * j + s - r
                        if 0 <= u <= 2:
                            wfcp[64 * kblk:64 * kblk + 64, par, v, j,
                                 64 * r:64 * r + 64] = ofs[:, :, u, v].T
    return to8(wd), to8(wfcp)


def _host_input(input):
    """sign -> fp8 -> per-pair patch-padded layout [core, pair, 128, g, p, 18, 20]."""
    x = np.asarray(input, np.float32)
    s = np.sign(x).astype(ml_dtypes.float8_e4m3fn)                 # [B, C, H, W]
    s = s.reshape(B, C, NP_SIDE, PATCH, NP_SIDE, PATCH).transpose(0, 1, 2, 4, 3, 5)
    xs = np.zeros((B, C, NP_SIDE, NP_SIDE, 18, 20), ml_dtypes.float8_e4m3fn)
    xs[..., 1:17, 2:18] = s
    # [B, C, ...] -> [core, pair, img(2), c, ...] -> partitions = img*64 + c
    xs = xs.reshape(N_CORES, N_PAIRS, 2 * C, NP_SIDE, NP_SIDE, 18, 20)
    return np.ascontiguousarray(xs)


def kernel(input, k, t, patch_filters, output_filters):
    global _CACHED_NC
    if _CACHED_NC is None:
        _CACHED_NC = _build_nc()
    nc = _CACHED_NC

    xs = _host_input(input)
    wd, wfcp = _host_weights(patch_filters, output_filters)
    in_maps = [
        {"xs": xs[i], "wd": wd, "wfcp": wfcp}
        for i in range(N_CORES)
    ]
    res = run_bass_kernel_spmd(nc, in_maps, list(range(N_CORES)))
    return np.concatenate(
        [r["y"] for r in res.results], axis=0
    ).astype(np.float32)
